# revision 24
# baseline (speedup 1.0000x reference)
"""Trainium2 Bass kernel for nn_Capsule: capsule layer with 3 dynamic-routing
iterations.

    u_hat = einsum('bip,iodp->biod', x, W)   # [64, 2048, 32, 32]
    3x routing: c = softmax(b, axis=2); s = sum_i c*u_hat; v = squash(s);
                b += sum_d v*u_hat

Strategy: shard in_caps (i) across 8 cores (256 each). W-shard and a
block-diagonalized x are SBUF-resident (bf16); u_hat is (re)computed on the
tensor engine each routing iteration, two capsules at a time, as
[K=32 (2i x 16p), M=128 (2i x 64b)] x [K=32, N=512 od] matmuls. Iteration 0
(uniform c) accumulates u directly in PSUM.

Iterations 1-2 process 64 "quads" (2 capsule pairs = 4 i each): u lands in
PSUM, is evacuated to SBUF bf16 (ACT), then per-quad the agreement products
m = u*v are computed on a per-quad route: "p8" = GPSIMD with fp8 output
(v pre-scaled x32), "d8" = DVE fp8, "d8" = DVE bf16. fp8 m tiles are
d-reduced on the PE with fp8 DoubleRow identity matmuls whose diagonal is
1/32 (un-scaling for free, half the PE cycles); bf16 m uses plain identity
accumulation. Agreements accumulate in a per-batch PSUM tile that feeds the
softmax (ACT exp + DVE reduce/reciprocal/normalize) and the logit store
(GPSIMD). c*u stays bf16 on DVE and is i-folded into PSUM via identity
matmuls one batch late. Per-core s partials are AllReduced after iters 0 and
1; the final iteration's partials are summed and squashed on the host.
"""

import numpy as np

B, IN_CAPS, IN_DIM = 64, 2048, 16
NUM_CAPS, DIM_CAPS = 32, 32
OD = NUM_CAPS * DIM_CAPS  # 1024
ROUTING_ITERS = 3
EPS = 1e-7

N_CORES = 8
I_LOC = IN_CAPS // N_CORES       # 256
N_PAIRS = I_LOC // 2             # 128
N_RG = 4                         # PE row groups
N_PJ = N_PAIRS // N_RG           # 32 pairs per row group
N_QUADS = N_PAIRS // 2           # 64
N_BATCH = N_QUADS // 4           # 16 softmax batches (4 quads each)

# Per-quad route for the agreement product m = u*v (pattern repeats /16):
#   p8: GPSIMD mult, fp8 out, PE DoubleRow fold (diag 1/32)
#   d8: DVE mult, fp8 out (1x mode), PE DoubleRow fold
#   d16: DVE mult, bf16 out (2x mode), PE bf16 identity fold
M_ROUTE_PAT = ["p8", "d16", "p8", "d8", "p8", "d16", "p8", "d16",
               "p8", "d8", "p8", "d16", "p8", "d16", "p8", "d8",
               "p8", "d16", "p8", "d16", "p8", "d8", "p8", "d16",
               "p8", "d16", "p8", "d8", "p8", "d16", "p8", "d8"]
# Per-pair evacuation engine for u (PSUM f32 -> SBUF bf16)
EVAC_PAT = ["act", "act", "act", "act", "act", "act", "act", "act",
            "act", "act", "act", "act", "act", "act", "act", "act"]

_CACHE = {}


def _build_nc(stage=3):
    # stage 3: full kernel; stage 4: timing variant (collectives skipped).
    import concourse.bacc as bacc
    import concourse.bass as bass
    import concourse.tile as tile
    from concourse import mybir

    f32 = mybir.dt.float32
    bf16 = mybir.dt.bfloat16
    f8 = mybir.dt.float8e4
    Alu = mybir.AluOpType
    Act = mybir.ActivationFunctionType
    AxX = mybir.AxisListType.X
    DR = mybir.MatmulPerfMode.DoubleRow

    nc = bacc.Bacc("TRN2", target_bir_lowering=False, debug=False,
                   num_devices=N_CORES)

    xbd_d = nc.dram_tensor("xbd", [128, N_PJ * 128], bf16,
                           kind="ExternalInput")
    wp_d = nc.dram_tensor("wp", [128, N_PJ * OD], bf16, kind="ExternalInput")
    out_d = nc.dram_tensor("out_sp2", [64, OD], f32,
                           kind="ExternalOutput")

    def lhsT_of(xbd, rg, pj):
        return xbd[32 * rg:32 * rg + 32, 128 * pj:128 * pj + 128]

    def rhs_of(wp, rg, pj, h):
        return wp[32 * rg:32 * rg + 32,
                  OD * pj + 512 * h:OD * pj + 512 * h + 512]

    with tile.TileContext(nc) as tc:
        with (
            nc.allow_low_precision(reason="bf16/fp8 routing intermediates"),
            tc.tile_pool(name="big", bufs=1) as big,
            tc.tile_pool(name="work", bufs=3) as work,
            tc.tile_pool(name="small", bufs=1) as small,
            tc.tile_pool(name="dram", bufs=1, space="DRAM") as dram,
        ):
            xbd = big.tile([128, N_PJ * 128], bf16)
            wp = big.tile([128, N_PJ * OD], bf16)
            # chunk the loads by pj-pairs so iteration 0 can start after the
            # first chunk lands instead of waiting ~25us for the full wp
            for g in range(16):
                pj0, pj1 = 2 * g, 2 * g + 2
                nc.sync.dma_start(xbd[:, 128 * pj0:128 * pj1],
                                  xbd_d[:, 128 * pj0:128 * pj1])
                nc.sync.dma_start(wp[:, OD * pj0:OD * pj1],
                                  wp_d[:, OD * pj0:OD * pj1])

            bl = big.tile([128, N_PAIRS * NUM_CAPS], bf16)  # routing logits
            vrep2 = big.tile([128, 2 * OD], bf16)   # v replicated (2p x 2f)
            vrep32_2 = big.tile([128, 2 * OD], bf16)  # 32*v (fp8 scaling)
            ident = big.tile([128, 128], bf16)              # PE-accumulate id
            from concourse.masks import make_identity
            make_identity(nc, ident[:])
            # fp8 DoubleRow identity: both 128-col halves = diag(1/32), so the
            # DR fold of (32*m) pairs yields the unscaled d-sum.
            identDR = big.tile([128, 256], f8)
            nc.gpsimd.memset(identDR[:], 0.0)
            for half in range(2):
                nc.gpsimd.affine_select(
                    out=identDR[:, 128 * half:128 * half + 128],
                    in_=identDR[:, 128 * half:128 * half + 128],
                    compare_op=Alu.not_equal, fill=1.0 / 32,
                    base=0, pattern=[[-1, 128]], channel_multiplier=1)
            identDR_ap = bass.AP(
                tensor=identDR.tensor, offset=identDR[:].offset,
                ap=[list(identDR[:].ap[0]), [128, 2], [1, 128]])
            # two-hot fold matrix: F[k, k % 64] = 1; cm-folds contract the
            # (i2) partition pair down to 64 batch rows for free
            foldF = big.tile([128, 64], bf16)
            nc.scalar.copy(out=foldF[0:64, :], in_=ident[0:64, 0:64])
            nc.scalar.copy(out=foldF[64:128, :], in_=ident[64:128, 64:128])
            eps_t = big.tile([64, 1], f32)
            nc.vector.memset(eps_t[:], EPS)

            ar_count = [0]

            # ---------------- iteration 0: s0 = (1/32) * sum_i u ----------
            with tc.tile_pool(name="ps0", bufs=1, space="PSUM") as ps0:
                acc0 = [ps0.tile([128, 512], f32, name=f"acc0_{h}",
                                 tag=f"acc0_{h}") for h in range(2)]
                for pj in range(N_PJ):
                    for h in range(2):
                        nc.tensor.matmul(
                            acc0[h][:],
                            xbd[:, 128 * pj:128 * pj + 128],
                            wp[:, OD * pj + 512 * h:OD * pj + 512 * h + 512],
                            start=(pj == 0), stop=(pj == N_PJ - 1),
                        )
                # fold the two capsule slots (partitions 0-63 + 64-127)
                s0 = small.tile([64, OD], f32, tag="sfold")
                tmpu = small.tile([64, OD], f32, tag="tmpu")
                for h in range(2):
                    th = work.tile([128, 512], f32, tag="thfold", bufs=2)
                    nc.scalar.copy(out=th[:], in_=acc0[h][:])
                    nc.sync.dma_start(tmpu[:, 512 * h:512 * h + 512],
                                      th[64:128, :])
                    nc.vector.tensor_add(out=s0[:, 512 * h:512 * h + 512],
                                         in0=th[0:64, :],
                                         in1=tmpu[:, 512 * h:512 * h + 512])
            nc.scalar.mul(out=s0[:], in_=s0[:], mul=1.0 / NUM_CAPS)

            def all_reduce(sp):
                if stage == 4:  # timing variant: skip collectives
                    return sp
                k = ar_count[0]
                ar_count[0] += 1
                ar_in = dram.tile([64, OD], f32, name=f"ar_in{k}",
                                  tag=f"ar_in{k}")
                ar_out = dram.tile([64, OD], f32, name=f"ar_out{k}",
                                   tag=f"ar_out{k}")
                nc.sync.dma_start(ar_in[:], sp[:])
                nc.gpsimd.collective_compute(
                    "AllReduce", Alu.add,
                    replica_groups=[list(range(N_CORES))],
                    ins=[ar_in.opt()], outs=[ar_out.opt()])
                sq = small.tile([64, OD], f32, tag="sfold")
                nc.sync.dma_start(sq[:], ar_out[:])
                return sq

            def squash_to_vrep(sq):
                """v = (n/(1+n)) * s / sqrt(n+eps), n = sum_d s^2; replicate
                v into both partition halves of vrep, and build vrep32."""
                ssq = small.tile([64, OD], f32, tag="tmpu")
                nc.vector.tensor_mul(out=ssq[:], in0=sq[:], in1=sq[:])
                n_t = small.tile([64, NUM_CAPS], f32, tag="n_t")
                nc.vector.tensor_reduce(
                    out=n_t[:],
                    in_=bass.AP(tensor=ssq.tensor, offset=ssq[:].offset,
                                ap=[list(ssq[:].ap[0]), [1, NUM_CAPS],
                                    [NUM_CAPS, DIM_CAPS]]),
                    axis=AxX, op=Alu.add)
                sr = small.tile([64, NUM_CAPS], f32, tag="sr")
                # sqrt via exp(0.5*ln): Ln/Exp share an ACT table set
                nc.scalar.activation(out=sr[:], in_=n_t[:], func=Act.Ln,
                                     bias=eps_t[:], scale=1.0)
                nc.scalar.activation(out=sr[:], in_=sr[:], func=Act.Exp,
                                     bias=0.0, scale=0.5)
                nc.vector.reciprocal(out=sr[:], in_=sr[:])   # 1/sqrt(n+eps)
                np1 = small.tile([64, NUM_CAPS], f32, tag="np1")
                nc.vector.tensor_scalar_add(out=np1[:], in0=n_t[:],
                                            scalar1=1.0)
                nc.vector.reciprocal(out=np1[:], in_=np1[:])  # 1/(1+n)
                fac = small.tile([64, NUM_CAPS], f32, tag="fac")
                nc.vector.tensor_mul(out=fac[:], in0=n_t[:], in1=np1[:])
                nc.vector.tensor_mul(out=fac[:], in0=fac[:], in1=sr[:])
                for half in range(2):
                    nc.vector.tensor_tensor(
                        out=vrep2[0:64, OD * half:OD * half + OD].rearrange(
                            "p (d o) -> p d o", d=DIM_CAPS),
                        in0=sq[:].rearrange("p (d o) -> p d o", d=DIM_CAPS),
                        in1=bass.AP(tensor=fac.tensor, offset=fac[:].offset,
                                    ap=[list(fac[:].ap[0]), [0, DIM_CAPS],
                                        [1, NUM_CAPS]]),
                        op=Alu.mult)
                nc.vector.tensor_scalar_mul(out=vrep32_2[0:64, :],
                                            in0=vrep2[0:64, :], scalar1=32.0)
                nc.sync.dma_start(vrep2[64:128, :], vrep2[0:64, :])
                nc.sync.dma_start(vrep32_2[64:128, :], vrep32_2[0:64, :])

            sq = all_reduce(s0)
            squash_to_vrep(sq)

            # ---------------- iterations 1..2 -----------------------------
            with tc.tile_pool(name="ps", bufs=2, space="PSUM") as ps:
                for it in range(1, ROUTING_ITERS):
                    accps = ps.tile([64, OD], f32, name="accps",
                                    tag="accps", bufs=1)
                    acc_started = [False, False]
                    cm_ready = []   # previous batch's cm tiles, safe to fold

                    def emit_cmacc(last=False):
                        for j, cmr in enumerate(cm_ready):
                            for s_ in range(2):
                                for h in range(2):
                                    st = not acc_started[h]
                                    acc_started[h] = True
                                    nc.tensor.matmul(
                                        accps[:, 512 * h:512 * h + 512],
                                        foldF[:],
                                        cmr[:, 1024 * s_ + 512 * h:
                                            1024 * s_ + 512 * h + 512],
                                        start=st,
                                        stop=(last and
                                              j == len(cm_ready) - 1 and
                                              s_ == 1),
                                        skip_group_check=True,
                                    )
                        cm_ready.clear()

                    def emit_agr_folds(st):
                        """d-reduce each m of a finished batch into agrps."""
                        agrps = st["agrps"]
                        # fold DVE-produced m first; GPSIMD m lands latest
                        for sub, m, route in sorted(
                                st["m"], key=lambda x: x[2] == "p8"):
                            for s_ in range(2):
                                osl = agrps[:, 64 * sub + 32 * s_:
                                            64 * sub + 32 * s_ + 32]
                                if route == "d16":
                                    for k in range(32):
                                        nc.tensor.matmul(
                                            osl, ident[:],
                                            m[:, 1024 * s_ + 32 * k:
                                              1024 * s_ + 32 * k + 32],
                                            start=(k == 0), stop=(k == 31),
                                            skip_group_check=True)
                                else:
                                    for k in range(16):
                                        rhs = bass.AP(
                                            tensor=m.tensor,
                                            offset=m[:, 1024 * s_ +
                                                     64 * k:].offset,
                                            ap=[list(m[:].ap[0]), [32, 2],
                                                [1, 32]])
                                        nc.tensor.matmul(
                                            osl, identDR_ap, rhs,
                                            start=(k == 0), stop=(k == 15),
                                            perf_mode=DR,
                                            skip_group_check=True)

                    def emit_softmax_head(st):
                        """blx + exp for a finished batch (emitted early so
                        the ACT queue reaches exp before the evac burst)."""
                        batch = st["batch"]
                        agrps = st["agrps"]
                        blsl = bl[:, 256 * batch:256 * (batch + 1)]
                        ce = work.tile([128, 256], bf16, tag="ce", bufs=2)
                        if it == 1:
                            # logits = agreement (b was zero)
                            nc.scalar.activation(out=ce[:], in_=agrps[:],
                                                 func=Act.Exp, bias=0.0,
                                                 scale=1.0)
                            nc.scalar.copy(out=blsl, in_=agrps[:])
                        else:
                            blx = work.tile([128, 256], bf16, tag="blx",
                                            bufs=2)
                            nc.vector.scalar_tensor_tensor(
                                out=blx[:], in0=agrps[:], scalar=1.0,
                                in1=blsl, op0=Alu.mult, op1=Alu.add)
                            nc.scalar.activation(out=ce[:], in_=blx[:],
                                                 func=Act.Exp, bias=0.0,
                                                 scale=1.0)
                        st["ce"] = ce

                    def emit_softmax_cm(st):
                        """Softmax tail + c*u for a finished batch; folds the
                        still-older batch's cm tiles on the PE."""
                        ce = st["ce"]
                        zs = work.tile([128, 8], f32, tag="zs", bufs=4)
                        nc.vector.tensor_reduce(
                            out=zs[:],
                            in_=ce[:].rearrange("p (s o) -> p s o", s=8),
                            axis=AxX, op=Alu.add)
                        zr = work.tile([128, 8], bf16, tag="zr", bufs=4)
                        nc.vector.reciprocal(out=zr[:], in_=zs[:])
                        nc.vector.tensor_tensor(
                            out=ce[:].rearrange("p (s o) -> p s o", s=8),
                            in0=ce[:].rearrange("p (s o) -> p s o", s=8),
                            in1=bass.AP(tensor=zr.tensor, offset=zr[:].offset,
                                        ap=[list(zr[:].ap[0]), [1, 8],
                                            [0, NUM_CAPS]]),
                            op=Alu.mult)
                        # fold the two-batches-ago c*u into PSUM now (the
                        # in-order PE never stalls on fresh cm)
                        emit_cmacc()
                        for sub, ub in enumerate(st["ub"]):
                            csl = bass.AP(
                                tensor=ce.tensor,
                                offset=ce[:, 64 * sub:].offset,
                                ap=[list(ce[:].ap[0]), [NUM_CAPS, 2],
                                    [0, DIM_CAPS], [1, NUM_CAPS]])
                            cm = work.tile([128, 2 * OD], bf16, name="cm",
                                           tag="cm", bufs=8)
                            nc.vector.tensor_tensor(
                                out=cm[:].rearrange("p (s d o) -> p s d o",
                                                    s=2, d=DIM_CAPS),
                                in0=ub[:].rearrange(
                                    "p (s d o) -> p s d o", s=2, d=DIM_CAPS),
                                in1=csl, op=Alu.mult)
                            cm_ready.append(cm)

                    prev = None
                    for batch in range(N_BATCH):
                        st = {"batch": batch, "ub": [], "m": [],
                              "agrps": ps.tile([128, 256], f32, name="agrps",
                                               tag="agrps", bufs=2)}
                        for sub in range(4):
                            q = 4 * batch + sub
                            route = M_ROUTE_PAT[q % 16]
                            ub = work.tile([128, 2 * OD], bf16, tag="ub",
                                           bufs=10)
                            for pr in range(2):
                                pair = 2 * q + pr
                                rg, pj = pair % N_RG, pair // N_RG
                                ups = ps.tile([128, OD], f32, name="ups",
                                              tag="ups", bufs=2)
                                for h in range(2):
                                    nc.tensor.matmul(
                                        ups[:, 512 * h:512 * h + 512],
                                        lhsT_of(xbd, rg, pj),
                                        rhs_of(wp, rg, pj, h),
                                        start=True, stop=True,
                                        tile_position=(32 * rg, 0),
                                    )
                                dst = ub[:, OD * pr:OD * (pr + 1)]
                                evac = EVAC_PAT[(2 * sub + pr) % 8]
                                if evac == "act":
                                    nc.scalar.copy(out=dst, in_=ups[:])
                                elif evac == "dve":
                                    nc.vector.tensor_copy(out=dst, in_=ups[:])
                                else:
                                    nc.gpsimd.tensor_copy(out=dst, in_=ups[:])
                            # agreement products m = u*v (or u*32v for fp8)
                            if route == "d16":
                                m = work.tile([128, 2 * OD], bf16, tag="m16",
                                              bufs=3)
                                nc.vector.tensor_tensor(
                                    out=m[:], in0=ub[:], in1=vrep2[:],
                                    op=Alu.mult)
                            elif route == "d8":
                                m = work.tile([128, 2 * OD], f8, tag="m8",
                                              bufs=8)
                                nc.vector.tensor_tensor(
                                    out=m[:], in0=ub[:], in1=vrep32_2[:],
                                    op=Alu.mult)
                            else:
                                m = work.tile([128, 2 * OD], f8, tag="m8",
                                              bufs=8)
                                nc.gpsimd.tensor_tensor(
                                    out=m[:], in0=ub[:], in1=vrep32_2[:],
                                    op=Alu.mult)
                            st["m"].append((sub, m, route))
                            st["ub"].append(ub)
                        if prev is not None:
                            emit_agr_folds(prev)
                            emit_softmax_head(prev)
                            emit_softmax_cm(prev)
                        prev = st
                    emit_agr_folds(prev)
                    emit_softmax_head(prev)
                    emit_softmax_cm(prev)
                    emit_cmacc(last=True)
                    # evacuate the PSUM s-partial (i2 already folded)
                    acc = work.tile([64, OD], f32, tag="accev", bufs=1)
                    nc.scalar.copy(out=acc[:], in_=accps[:])
                    if it < ROUTING_ITERS - 1:
                        sq = all_reduce(acc)
                        squash_to_vrep(sq)
                    else:
                        nc.sync.dma_start(out_d[:], acc[:])
    nc.compile()
    return nc


def _prep_inputs(x, W):
    """Build per-core xbd [128, N_PJ*128] and wp [128, N_PJ*OD] arrays."""
    import ml_dtypes
    bf16 = ml_dtypes.bfloat16
    ins = []
    for c in range(N_CORES):
        xc = x[:, c * I_LOC:(c + 1) * I_LOC, :]          # [64, 256, 16]
        Wc = W[c * I_LOC:(c + 1) * I_LOC]                # [256, 32, 32, 16]
        # i_loc = 8*pj + 2*rg + ipar
        xr = np.ascontiguousarray(
            xc.reshape(B, N_PJ, N_RG, 2, IN_DIM)
              .transpose(3, 2, 4, 1, 0))                 # [ipar,rg,p,pj,b]
        xbd = np.zeros((N_RG, 2, IN_DIM, N_PJ, 2, B), dtype=np.float32)
        xbd[:, 0, :, :, 0, :] = xr[0]
        xbd[:, 1, :, :, 1, :] = xr[1]
        xbd = xbd.reshape(128, N_PJ * 128).astype(bf16)
        wr = np.ascontiguousarray(
            Wc.reshape(N_PJ, N_RG, 2, NUM_CAPS, DIM_CAPS, IN_DIM)
              .transpose(1, 2, 5, 0, 4, 3)               # [rg,ipar,p,pj,d,o]
              .reshape(128, N_PJ * OD)).astype(bf16)
        ins.append({"xbd": xbd, "wp": wr})
    return ins


def _squash_np(s):
    n = np.sum(np.square(s), axis=-1, keepdims=True)
    return (n / (1.0 + n)) * (s / np.sqrt(n + EPS))


def kernel(x, W, _trace=False):
    from concourse.bass_utils import run_bass_kernel_spmd

    x = np.asarray(x, dtype=np.float32)
    W = np.asarray(W, dtype=np.float32)
    if "nc" not in _CACHE:
        _CACHE["nc"] = _build_nc()
    nc = _CACHE["nc"]
    in_maps = _prep_inputs(x, W)
    res = run_bass_kernel_spmd(nc, in_maps, core_ids=list(range(N_CORES)),
                               trace=_trace)
    _CACHE["last_result"] = res
    sp = np.stack([r["out_sp2"] for r in res.results])   # [8, 64, OD]
    s2 = sp.sum(axis=0)
    s2_od = s2.reshape(B, DIM_CAPS, NUM_CAPS).transpose(0, 2, 1)
    v = _squash_np(np.ascontiguousarray(s2_od))
    return v.astype(np.float32)


# revision 26
# speedup vs baseline: 1.0565x; 1.0565x over previous
"""Trainium2 Bass kernel for nn_Capsule: capsule layer with 3 dynamic-routing
iterations.

    u_hat = einsum('bip,iodp->biod', x, W)   # [64, 2048, 32, 32]
    3x routing: c = softmax(b, axis=2); s = sum_i c*u_hat; v = squash(s);
                b += sum_d v*u_hat

Strategy: shard in_caps (i) across 8 cores (256 each). W-shard and a
block-diagonalized x are SBUF-resident (bf16); u_hat is (re)computed on the
tensor engine each routing iteration, two capsules at a time, as
[K=32 (2i x 16p), M=128 (2i x 64b)] x [K=32, N=512 od] matmuls. Iteration 0
(uniform c) accumulates u directly in PSUM.

Iterations 1-2 process 64 "quads" (2 capsule pairs = 4 i each): u lands in
PSUM, is evacuated to SBUF bf16 (ACT), then per-quad the agreement products
m = u*v are computed on a per-quad route: "p8" = GPSIMD with fp8 output
(v pre-scaled x32), "d8" = DVE fp8, "d8" = DVE bf16. fp8 m tiles are
d-reduced on the PE with fp8 DoubleRow identity matmuls whose diagonal is
1/32 (un-scaling for free, half the PE cycles); bf16 m uses plain identity
accumulation. Agreements accumulate in a per-batch PSUM tile that feeds the
softmax (ACT exp + DVE reduce/reciprocal/normalize) and the logit store
(GPSIMD). c*u stays bf16 on DVE and is i-folded into PSUM via identity
matmuls one batch late. Per-core s partials are AllReduced after iters 0 and
1; the final iteration's partials are summed and squashed on the host.
"""

import numpy as np

B, IN_CAPS, IN_DIM = 64, 2048, 16
NUM_CAPS, DIM_CAPS = 32, 32
OD = NUM_CAPS * DIM_CAPS  # 1024
ROUTING_ITERS = 3
EPS = 1e-7

N_CORES = 8
I_LOC = IN_CAPS // N_CORES       # 256
N_PAIRS = I_LOC // 2             # 128
N_RG = 4                         # PE row groups
N_PJ = N_PAIRS // N_RG           # 32 pairs per row group
N_QUADS = N_PAIRS // 2           # 64
N_BATCH = N_QUADS // 4           # 16 softmax batches (4 quads each)

# Per-quad route for the agreement product m = u*v (pattern repeats /16):
#   p8: GPSIMD mult, fp8 out, PE DoubleRow fold (diag 1/32)
#   d8: DVE mult, fp8 out (1x mode), PE DoubleRow fold
#   d16: DVE mult, bf16 out (2x mode), PE bf16 identity fold
M_ROUTE_PAT = ["p8", "d16", "p8", "d16", "p8", "d16", "p8", "d16",
               "p8", "d16", "p8", "d16", "p8", "d16", "p8", "d16",
               "p8", "d16", "p8", "d16", "p8", "d16", "p8", "d16",
               "p8", "d16", "p8", "d16", "p8", "d16", "p8", "d16"]
# Per-pair evacuation engine for u (PSUM f32 -> SBUF bf16)
EVAC_PAT = ["act", "act", "act", "act", "act", "act", "act", "act",
            "act", "act", "act", "act", "act", "act", "act", "act"]

_CACHE = {}


def _build_nc(stage=3):
    # stage 3: full kernel; stage 4: timing variant (collectives skipped).
    import concourse.bacc as bacc
    import concourse.bass as bass
    import concourse.tile as tile
    from concourse import mybir

    f32 = mybir.dt.float32
    bf16 = mybir.dt.bfloat16
    f8 = mybir.dt.float8e4
    Alu = mybir.AluOpType
    Act = mybir.ActivationFunctionType
    AxX = mybir.AxisListType.X
    DR = mybir.MatmulPerfMode.DoubleRow

    nc = bacc.Bacc("TRN2", target_bir_lowering=False, debug=False,
                   num_devices=N_CORES)

    xbd_d = nc.dram_tensor("xbd", [128, N_PJ * 128], bf16,
                           kind="ExternalInput")
    wp_d = nc.dram_tensor("wp", [128, N_PJ * OD], bf16, kind="ExternalInput")
    out_d = nc.dram_tensor("out_sp2", [64, OD], f32,
                           kind="ExternalOutput")

    def lhsT_of(xbd, rg, pj):
        return xbd[32 * rg:32 * rg + 32, 128 * pj:128 * pj + 128]

    def rhs_of(wp, rg, pj, h):
        return wp[32 * rg:32 * rg + 32,
                  OD * pj + 512 * h:OD * pj + 512 * h + 512]

    with tile.TileContext(nc) as tc:
        with (
            nc.allow_low_precision(reason="bf16/fp8 routing intermediates"),
            tc.tile_pool(name="big", bufs=1) as big,
            tc.tile_pool(name="work", bufs=3) as work,
            tc.tile_pool(name="small", bufs=1) as small,
            tc.tile_pool(name="dram", bufs=1, space="DRAM") as dram,
        ):
            xbd = big.tile([128, N_PJ * 128], bf16)
            wp = big.tile([128, N_PJ * OD], bf16)
            # chunk the loads by pj-pairs so iteration 0 can start after the
            # first chunk lands instead of waiting ~25us for the full wp
            for g in range(16):
                pj0, pj1 = 2 * g, 2 * g + 2
                nc.sync.dma_start(xbd[:, 128 * pj0:128 * pj1],
                                  xbd_d[:, 128 * pj0:128 * pj1])
                nc.sync.dma_start(wp[:, OD * pj0:OD * pj1],
                                  wp_d[:, OD * pj0:OD * pj1])

            bl = big.tile([128, N_PAIRS * NUM_CAPS], bf16)  # routing logits
            vrep2 = big.tile([128, 2 * OD], bf16)   # v replicated (2p x 2f)
            vrep32_2 = big.tile([128, 2 * OD], bf16)  # 32*v (fp8 scaling)
            ident = big.tile([128, 128], bf16)              # PE-accumulate id
            from concourse.masks import make_identity
            make_identity(nc, ident[:])
            # fp8 DoubleRow identity: both 128-col halves = diag(1/32), so the
            # DR fold of (32*m) pairs yields the unscaled d-sum.
            identDR = big.tile([128, 256], f8)
            nc.gpsimd.memset(identDR[:], 0.0)
            for half in range(2):
                nc.gpsimd.affine_select(
                    out=identDR[:, 128 * half:128 * half + 128],
                    in_=identDR[:, 128 * half:128 * half + 128],
                    compare_op=Alu.not_equal, fill=1.0 / 32,
                    base=0, pattern=[[-1, 128]], channel_multiplier=1)
            identDR_ap = bass.AP(
                tensor=identDR.tensor, offset=identDR[:].offset,
                ap=[list(identDR[:].ap[0]), [128, 2], [1, 128]])
            # two-hot fold matrix: F[k, k % 64] = 1; cm-folds contract the
            # (i2) partition pair down to 64 batch rows for free
            foldF = big.tile([128, 64], bf16)
            nc.scalar.copy(out=foldF[0:64, :], in_=ident[0:64, 0:64])
            nc.scalar.copy(out=foldF[64:128, :], in_=ident[64:128, 64:128])
            eps_t = big.tile([64, 1], f32)
            nc.vector.memset(eps_t[:], EPS)

            # DRAM staging for u: iteration 2 reloads the evacuated u tiles
            # instead of recomputing (PE) and re-evacuating (ACT) them
            u_dram = [dram.tile([128, 2 * OD], bf16, name=f"uq{q}",
                                tag=f"uq{q}") for q in range(N_QUADS)]

            ar_count = [0]

            # ---------------- iteration 0: s0 = (1/32) * sum_i u ----------
            with tc.tile_pool(name="ps0", bufs=1, space="PSUM") as ps0:
                acc0 = [ps0.tile([128, 512], f32, name=f"acc0_{h}",
                                 tag=f"acc0_{h}") for h in range(2)]
                for pj in range(N_PJ):
                    for h in range(2):
                        nc.tensor.matmul(
                            acc0[h][:],
                            xbd[:, 128 * pj:128 * pj + 128],
                            wp[:, OD * pj + 512 * h:OD * pj + 512 * h + 512],
                            start=(pj == 0), stop=(pj == N_PJ - 1),
                        )
                # fold the two capsule slots (partitions 0-63 + 64-127)
                s0 = small.tile([64, OD], f32, tag="sfold")
                tmpu = small.tile([64, OD], f32, tag="tmpu")
                for h in range(2):
                    th = work.tile([128, 512], f32, tag="thfold", bufs=2)
                    nc.scalar.copy(out=th[:], in_=acc0[h][:])
                    nc.sync.dma_start(tmpu[:, 512 * h:512 * h + 512],
                                      th[64:128, :])
                    nc.vector.tensor_add(out=s0[:, 512 * h:512 * h + 512],
                                         in0=th[0:64, :],
                                         in1=tmpu[:, 512 * h:512 * h + 512])
            nc.scalar.mul(out=s0[:], in_=s0[:], mul=1.0 / NUM_CAPS)

            def all_reduce(sp):
                if stage == 4:  # timing variant: skip collectives
                    return sp
                k = ar_count[0]
                ar_count[0] += 1
                ar_in = dram.tile([64, OD], f32, name=f"ar_in{k}",
                                  tag=f"ar_in{k}")
                ar_out = dram.tile([64, OD], f32, name=f"ar_out{k}",
                                   tag=f"ar_out{k}")
                nc.sync.dma_start(ar_in[:], sp[:])
                nc.gpsimd.collective_compute(
                    "AllReduce", Alu.add,
                    replica_groups=[list(range(N_CORES))],
                    ins=[ar_in.opt()], outs=[ar_out.opt()])
                sq = small.tile([64, OD], f32, tag="sfold")
                nc.sync.dma_start(sq[:], ar_out[:])
                return sq

            def squash_to_vrep(sq):
                """v = (n/(1+n)) * s / sqrt(n+eps), n = sum_d s^2; replicate
                v into both partition halves of vrep, and build vrep32."""
                ssq = small.tile([64, OD], f32, tag="tmpu")
                nc.vector.tensor_mul(out=ssq[:], in0=sq[:], in1=sq[:])
                n_t = small.tile([64, NUM_CAPS], f32, tag="n_t")
                nc.vector.tensor_reduce(
                    out=n_t[:],
                    in_=bass.AP(tensor=ssq.tensor, offset=ssq[:].offset,
                                ap=[list(ssq[:].ap[0]), [1, NUM_CAPS],
                                    [NUM_CAPS, DIM_CAPS]]),
                    axis=AxX, op=Alu.add)
                sr = small.tile([64, NUM_CAPS], f32, tag="sr")
                # sqrt via exp(0.5*ln): Ln/Exp share an ACT table set
                nc.scalar.activation(out=sr[:], in_=n_t[:], func=Act.Ln,
                                     bias=eps_t[:], scale=1.0)
                nc.scalar.activation(out=sr[:], in_=sr[:], func=Act.Exp,
                                     bias=0.0, scale=0.5)
                nc.vector.reciprocal(out=sr[:], in_=sr[:])   # 1/sqrt(n+eps)
                np1 = small.tile([64, NUM_CAPS], f32, tag="np1")
                nc.vector.tensor_scalar_add(out=np1[:], in0=n_t[:],
                                            scalar1=1.0)
                nc.vector.reciprocal(out=np1[:], in_=np1[:])  # 1/(1+n)
                fac = small.tile([64, NUM_CAPS], f32, tag="fac")
                nc.vector.tensor_mul(out=fac[:], in0=n_t[:], in1=np1[:])
                nc.vector.tensor_mul(out=fac[:], in0=fac[:], in1=sr[:])
                for half in range(2):
                    nc.vector.tensor_tensor(
                        out=vrep2[0:64, OD * half:OD * half + OD].rearrange(
                            "p (d o) -> p d o", d=DIM_CAPS),
                        in0=sq[:].rearrange("p (d o) -> p d o", d=DIM_CAPS),
                        in1=bass.AP(tensor=fac.tensor, offset=fac[:].offset,
                                    ap=[list(fac[:].ap[0]), [0, DIM_CAPS],
                                        [1, NUM_CAPS]]),
                        op=Alu.mult)
                nc.vector.tensor_scalar_mul(out=vrep32_2[0:64, :],
                                            in0=vrep2[0:64, :], scalar1=32.0)
                nc.sync.dma_start(vrep2[64:128, :], vrep2[0:64, :])
                nc.sync.dma_start(vrep32_2[64:128, :], vrep32_2[0:64, :])

            sq = all_reduce(s0)
            squash_to_vrep(sq)

            # ---------------- iterations 1..2 -----------------------------
            with tc.tile_pool(name="ps", bufs=2, space="PSUM") as ps:
                for it in range(1, ROUTING_ITERS):
                    accps = ps.tile([64, OD], f32, name="accps",
                                    tag="accps", bufs=1)
                    acc_started = [False, False]
                    cm_ready = []   # previous batch's cm tiles, safe to fold

                    def emit_cmacc(last=False):
                        for j, cmr in enumerate(cm_ready):
                            for s_ in range(2):
                                for h in range(2):
                                    st = not acc_started[h]
                                    acc_started[h] = True
                                    nc.tensor.matmul(
                                        accps[:, 512 * h:512 * h + 512],
                                        foldF[:],
                                        cmr[:, 1024 * s_ + 512 * h:
                                            1024 * s_ + 512 * h + 512],
                                        start=st,
                                        stop=(last and
                                              j == len(cm_ready) - 1 and
                                              s_ == 1),
                                        skip_group_check=True,
                                    )
                        cm_ready.clear()

                    def emit_agr_folds(st):
                        """d-reduce each m of a finished batch into agrps."""
                        agrps = st["agrps"]
                        # fold DVE-produced m first; GPSIMD m lands latest
                        for sub, m, route in sorted(
                                st["m"], key=lambda x: x[2] == "p8"):
                            for s_ in range(2):
                                osl = agrps[:, 64 * sub + 32 * s_:
                                            64 * sub + 32 * s_ + 32]
                                if route == "d16":
                                    for k in range(32):
                                        nc.tensor.matmul(
                                            osl, ident[:],
                                            m[:, 1024 * s_ + 32 * k:
                                              1024 * s_ + 32 * k + 32],
                                            start=(k == 0), stop=(k == 31),
                                            skip_group_check=True)
                                else:
                                    for k in range(16):
                                        rhs = bass.AP(
                                            tensor=m.tensor,
                                            offset=m[:, 1024 * s_ +
                                                     64 * k:].offset,
                                            ap=[list(m[:].ap[0]), [32, 2],
                                                [1, 32]])
                                        nc.tensor.matmul(
                                            osl, identDR_ap, rhs,
                                            start=(k == 0), stop=(k == 15),
                                            perf_mode=DR,
                                            skip_group_check=True)

                    def emit_softmax_head(st):
                        """blx + exp for a finished batch (emitted early so
                        the ACT queue reaches exp before the evac burst)."""
                        batch = st["batch"]
                        agrps = st["agrps"]
                        blsl = bl[:, 256 * batch:256 * (batch + 1)]
                        ce = work.tile([128, 256], bf16, tag="ce", bufs=2)
                        if it == 1:
                            # logits = agreement (b was zero)
                            nc.scalar.activation(out=ce[:], in_=agrps[:],
                                                 func=Act.Exp, bias=0.0,
                                                 scale=1.0)
                            nc.scalar.copy(out=blsl, in_=agrps[:])
                        else:
                            blx = work.tile([128, 256], bf16, tag="blx",
                                            bufs=2)
                            nc.vector.scalar_tensor_tensor(
                                out=blx[:], in0=agrps[:], scalar=1.0,
                                in1=blsl, op0=Alu.mult, op1=Alu.add)
                            nc.scalar.activation(out=ce[:], in_=blx[:],
                                                 func=Act.Exp, bias=0.0,
                                                 scale=1.0)
                        st["ce"] = ce

                    def emit_softmax_cm(st):
                        """Softmax tail + c*u for a finished batch; folds the
                        still-older batch's cm tiles on the PE."""
                        ce = st["ce"]
                        zs = work.tile([128, 8], f32, tag="zs", bufs=4)
                        nc.vector.tensor_reduce(
                            out=zs[:],
                            in_=ce[:].rearrange("p (s o) -> p s o", s=8),
                            axis=AxX, op=Alu.add)
                        zr = work.tile([128, 8], bf16, tag="zr", bufs=4)
                        nc.vector.reciprocal(out=zr[:], in_=zs[:])
                        nc.vector.tensor_tensor(
                            out=ce[:].rearrange("p (s o) -> p s o", s=8),
                            in0=ce[:].rearrange("p (s o) -> p s o", s=8),
                            in1=bass.AP(tensor=zr.tensor, offset=zr[:].offset,
                                        ap=[list(zr[:].ap[0]), [1, 8],
                                            [0, NUM_CAPS]]),
                            op=Alu.mult)
                        # fold the two-batches-ago c*u into PSUM now (the
                        # in-order PE never stalls on fresh cm)
                        emit_cmacc()
                        for sub, ub in enumerate(st["ub"]):
                            csl = bass.AP(
                                tensor=ce.tensor,
                                offset=ce[:, 64 * sub:].offset,
                                ap=[list(ce[:].ap[0]), [NUM_CAPS, 2],
                                    [0, DIM_CAPS], [1, NUM_CAPS]])
                            cm = work.tile([128, 2 * OD], bf16, name="cm",
                                           tag="cm", bufs=8)
                            nc.vector.tensor_tensor(
                                out=cm[:].rearrange("p (s d o) -> p s d o",
                                                    s=2, d=DIM_CAPS),
                                in0=ub[:].rearrange(
                                    "p (s d o) -> p s d o", s=2, d=DIM_CAPS),
                                in1=csl, op=Alu.mult)
                            cm_ready.append(cm)

                    prev = None
                    for batch in range(N_BATCH):
                        st = {"batch": batch, "ub": [], "m": [],
                              "agrps": ps.tile([128, 256], f32, name="agrps",
                                               tag="agrps", bufs=2)}
                        for sub in range(4):
                            q = 4 * batch + sub
                            route = M_ROUTE_PAT[q % 16]
                            ub = work.tile([128, 2 * OD], bf16, tag="ub",
                                           bufs=10)
                            if it > 1:
                                # u is bit-identical across iterations:
                                # reload it from DRAM instead of recomputing
                                # on the PE and re-evacuating through ACT
                                nc.sync.dma_start(ub[:], u_dram[q][:])
                            else:
                                for pr in range(2):
                                    pair = 2 * q + pr
                                    rg, pj = pair % N_RG, pair // N_RG
                                    ups = ps.tile([128, OD], f32, name="ups",
                                                  tag="ups", bufs=2)
                                    for h in range(2):
                                        nc.tensor.matmul(
                                            ups[:, 512 * h:512 * h + 512],
                                            lhsT_of(xbd, rg, pj),
                                            rhs_of(wp, rg, pj, h),
                                            start=True, stop=True,
                                            tile_position=(32 * rg, 0),
                                        )
                                    dst = ub[:, OD * pr:OD * (pr + 1)]
                                    evac = EVAC_PAT[(2 * sub + pr) % 8]
                                    if evac == "act":
                                        nc.scalar.copy(out=dst, in_=ups[:])
                                    elif evac == "dve":
                                        nc.vector.tensor_copy(out=dst,
                                                              in_=ups[:])
                                    else:
                                        nc.gpsimd.tensor_copy(out=dst,
                                                              in_=ups[:])
                                nc.sync.dma_start(u_dram[q][:], ub[:])
                            # agreement products m = u*v (or u*32v for fp8)
                            if route == "d16":
                                m = work.tile([128, 2 * OD], bf16, tag="m16",
                                              bufs=3)
                                nc.vector.tensor_tensor(
                                    out=m[:], in0=ub[:], in1=vrep2[:],
                                    op=Alu.mult)
                            elif route == "d8":
                                m = work.tile([128, 2 * OD], f8, tag="m8",
                                              bufs=8)
                                nc.vector.tensor_tensor(
                                    out=m[:], in0=ub[:], in1=vrep32_2[:],
                                    op=Alu.mult)
                            else:
                                m = work.tile([128, 2 * OD], f8, tag="m8",
                                              bufs=8)
                                nc.gpsimd.tensor_tensor(
                                    out=m[:], in0=ub[:], in1=vrep32_2[:],
                                    op=Alu.mult)
                            st["m"].append((sub, m, route))
                            st["ub"].append(ub)
                        if prev is not None:
                            emit_agr_folds(prev)
                            emit_softmax_head(prev)
                            emit_softmax_cm(prev)
                        prev = st
                    emit_agr_folds(prev)
                    emit_softmax_head(prev)
                    emit_softmax_cm(prev)
                    emit_cmacc(last=True)
                    # evacuate the PSUM s-partial (i2 already folded)
                    acc = work.tile([64, OD], f32, tag="accev", bufs=1)
                    nc.scalar.copy(out=acc[:], in_=accps[:])
                    if it < ROUTING_ITERS - 1:
                        sq = all_reduce(acc)
                        squash_to_vrep(sq)
                    else:
                        nc.sync.dma_start(out_d[:], acc[:])
    nc.compile()
    return nc


def _prep_inputs(x, W):
    """Build per-core xbd [128, N_PJ*128] and wp [128, N_PJ*OD] arrays."""
    import ml_dtypes
    bf16 = ml_dtypes.bfloat16
    ins = []
    for c in range(N_CORES):
        xc = x[:, c * I_LOC:(c + 1) * I_LOC, :]          # [64, 256, 16]
        Wc = W[c * I_LOC:(c + 1) * I_LOC]                # [256, 32, 32, 16]
        # i_loc = 8*pj + 2*rg + ipar
        xr = np.ascontiguousarray(
            xc.reshape(B, N_PJ, N_RG, 2, IN_DIM)
              .transpose(3, 2, 4, 1, 0))                 # [ipar,rg,p,pj,b]
        xbd = np.zeros((N_RG, 2, IN_DIM, N_PJ, 2, B), dtype=np.float32)
        xbd[:, 0, :, :, 0, :] = xr[0]
        xbd[:, 1, :, :, 1, :] = xr[1]
        xbd = xbd.reshape(128, N_PJ * 128).astype(bf16)
        wr = np.ascontiguousarray(
            Wc.reshape(N_PJ, N_RG, 2, NUM_CAPS, DIM_CAPS, IN_DIM)
              .transpose(1, 2, 5, 0, 4, 3)               # [rg,ipar,p,pj,d,o]
              .reshape(128, N_PJ * OD)).astype(bf16)
        ins.append({"xbd": xbd, "wp": wr})
    return ins


def _squash_np(s):
    n = np.sum(np.square(s), axis=-1, keepdims=True)
    return (n / (1.0 + n)) * (s / np.sqrt(n + EPS))


def kernel(x, W, _trace=False):
    from concourse.bass_utils import run_bass_kernel_spmd

    x = np.asarray(x, dtype=np.float32)
    W = np.asarray(W, dtype=np.float32)
    if "nc" not in _CACHE:
        _CACHE["nc"] = _build_nc()
    nc = _CACHE["nc"]
    in_maps = _prep_inputs(x, W)
    res = run_bass_kernel_spmd(nc, in_maps, core_ids=list(range(N_CORES)),
                               trace=_trace)
    _CACHE["last_result"] = res
    sp = np.stack([r["out_sp2"] for r in res.results])   # [8, 64, OD]
    s2 = sp.sum(axis=0)
    s2_od = s2.reshape(B, DIM_CAPS, NUM_CAPS).transpose(0, 2, 1)
    v = _squash_np(np.ascontiguousarray(s2_od))
    return v.astype(np.float32)


# revision 27
# speedup vs baseline: 1.0653x; 1.0084x over previous
"""Trainium2 Bass kernel for nn_Capsule: capsule layer with 3 dynamic-routing
iterations.

    u_hat = einsum('bip,iodp->biod', x, W)   # [64, 2048, 32, 32]
    3x routing: c = softmax(b, axis=2); s = sum_i c*u_hat; v = squash(s);
                b += sum_d v*u_hat

Strategy: shard in_caps (i) across 8 cores (256 each). W-shard and a
block-diagonalized x are SBUF-resident (bf16); u_hat is (re)computed on the
tensor engine each routing iteration, two capsules at a time, as
[K=32 (2i x 16p), M=128 (2i x 64b)] x [K=32, N=512 od] matmuls. Iteration 0
(uniform c) accumulates u directly in PSUM.

Iterations 1-2 process 64 "quads" (2 capsule pairs = 4 i each): u lands in
PSUM, is evacuated to SBUF bf16 (ACT), then per-quad the agreement products
m = u*v are computed on a per-quad route: "p8" = GPSIMD with fp8 output
(v pre-scaled x32), "d8" = DVE fp8, "d8" = DVE bf16. fp8 m tiles are
d-reduced on the PE with fp8 DoubleRow identity matmuls whose diagonal is
1/32 (un-scaling for free, half the PE cycles); bf16 m uses plain identity
accumulation. Agreements accumulate in a per-batch PSUM tile that feeds the
softmax (ACT exp + DVE reduce/reciprocal/normalize) and the logit store
(GPSIMD). c*u stays bf16 on DVE and is i-folded into PSUM via identity
matmuls one batch late. Per-core s partials are AllReduced after iters 0 and
1; the final iteration's partials are summed and squashed on the host.
"""

import numpy as np

B, IN_CAPS, IN_DIM = 64, 2048, 16
NUM_CAPS, DIM_CAPS = 32, 32
OD = NUM_CAPS * DIM_CAPS  # 1024
ROUTING_ITERS = 3
EPS = 1e-7

N_CORES = 8
I_LOC = IN_CAPS // N_CORES       # 256
N_PAIRS = I_LOC // 2             # 128
N_RG = 4                         # PE row groups
N_PJ = N_PAIRS // N_RG           # 32 pairs per row group
N_QUADS = N_PAIRS // 2           # 64
N_BATCH = N_QUADS // 4           # 16 softmax batches (4 quads each)

# Per-quad route for the agreement product m = u*v (pattern repeats /16):
#   p8: GPSIMD mult, fp8 out, PE DoubleRow fold (diag 1/32)
#   d8: DVE mult, fp8 out (1x mode), PE DoubleRow fold
#   d16: DVE mult, bf16 out (2x mode), PE bf16 identity fold
M_ROUTE_PAT = ["p8", "d16", "p8", "d16", "p8", "d16", "p8", "d16",
               "p8", "d16", "p8", "d16", "p8", "d16", "p8", "d16",
               "p8", "d16", "p8", "d16", "p8", "d16", "p8", "d16",
               "p8", "d16", "p8", "d16", "p8", "d16", "p8", "d16"]
# Per-pair evacuation engine for u (PSUM f32 -> SBUF bf16)
EVAC_PAT = ["act", "act", "act", "dve", "act", "act", "act", "act",
            "act", "act", "act", "dve", "act", "act", "act", "act"]

_CACHE = {}


def _build_nc(stage=3):
    # stage 3: full kernel; stage 4: timing variant (collectives skipped).
    import concourse.bacc as bacc
    import concourse.bass as bass
    import concourse.tile as tile
    from concourse import mybir

    f32 = mybir.dt.float32
    bf16 = mybir.dt.bfloat16
    f8 = mybir.dt.float8e4
    Alu = mybir.AluOpType
    Act = mybir.ActivationFunctionType
    AxX = mybir.AxisListType.X
    DR = mybir.MatmulPerfMode.DoubleRow

    nc = bacc.Bacc("TRN2", target_bir_lowering=False, debug=False,
                   num_devices=N_CORES)

    xbd_d = nc.dram_tensor("xbd", [128, N_PJ * 128], bf16,
                           kind="ExternalInput")
    wp_d = nc.dram_tensor("wp", [128, N_PJ * OD], bf16, kind="ExternalInput")
    out_d = nc.dram_tensor("out_sp2", [64, OD], f32,
                           kind="ExternalOutput")

    def lhsT_of(xbd, rg, pj):
        return xbd[32 * rg:32 * rg + 32, 128 * pj:128 * pj + 128]

    def rhs_of(wp, rg, pj, h):
        return wp[32 * rg:32 * rg + 32,
                  OD * pj + 512 * h:OD * pj + 512 * h + 512]

    with tile.TileContext(nc) as tc:
        with (
            nc.allow_low_precision(reason="bf16/fp8 routing intermediates"),
            tc.tile_pool(name="big", bufs=1) as big,
            tc.tile_pool(name="work", bufs=3) as work,
            tc.tile_pool(name="small", bufs=1) as small,
            tc.tile_pool(name="dram", bufs=1, space="DRAM") as dram,
        ):
            xbd = big.tile([128, N_PJ * 128], bf16)
            wp = big.tile([128, N_PJ * OD], bf16)
            # chunk the loads by pj-pairs so iteration 0 can start after the
            # first chunk lands instead of waiting ~25us for the full wp
            for g in range(16):
                pj0, pj1 = 2 * g, 2 * g + 2
                nc.sync.dma_start(xbd[:, 128 * pj0:128 * pj1],
                                  xbd_d[:, 128 * pj0:128 * pj1])
                nc.sync.dma_start(wp[:, OD * pj0:OD * pj1],
                                  wp_d[:, OD * pj0:OD * pj1])

            bl = big.tile([128, N_PAIRS * NUM_CAPS], bf16)  # routing logits
            vrep2 = big.tile([128, 2 * OD], bf16)   # v replicated (2p x 2f)
            vrep32_2 = big.tile([128, 2 * OD], bf16)  # 32*v (fp8 scaling)
            ident = big.tile([128, 128], bf16)              # PE-accumulate id
            from concourse.masks import make_identity
            make_identity(nc, ident[:])
            # fp8 DoubleRow identity: both 128-col halves = diag(1/32), so the
            # DR fold of (32*m) pairs yields the unscaled d-sum.
            identDR = big.tile([128, 256], f8)
            nc.gpsimd.memset(identDR[:], 0.0)
            for half in range(2):
                nc.gpsimd.affine_select(
                    out=identDR[:, 128 * half:128 * half + 128],
                    in_=identDR[:, 128 * half:128 * half + 128],
                    compare_op=Alu.not_equal, fill=1.0 / 32,
                    base=0, pattern=[[-1, 128]], channel_multiplier=1)
            identDR_ap = bass.AP(
                tensor=identDR.tensor, offset=identDR[:].offset,
                ap=[list(identDR[:].ap[0]), [128, 2], [1, 128]])
            # two-hot fold matrix: F[k, k % 64] = 1; cm-folds contract the
            # (i2) partition pair down to 64 batch rows for free
            foldF = big.tile([128, 64], bf16)
            nc.scalar.copy(out=foldF[0:64, :], in_=ident[0:64, 0:64])
            nc.scalar.copy(out=foldF[64:128, :], in_=ident[64:128, 64:128])
            eps_t = big.tile([64, 1], f32)
            nc.vector.memset(eps_t[:], EPS)

            # DRAM staging for u: iteration 2 reloads the evacuated u tiles
            # instead of recomputing (PE) and re-evacuating (ACT) them
            u_dram = [dram.tile([128, 2 * OD], bf16, name=f"uq{q}",
                                tag=f"uq{q}") for q in range(N_QUADS)]

            ar_count = [0]

            # ---------------- iteration 0: s0 = (1/32) * sum_i u ----------
            with tc.tile_pool(name="ps0", bufs=1, space="PSUM") as ps0:
                acc0 = [ps0.tile([128, 512], f32, name=f"acc0_{h}",
                                 tag=f"acc0_{h}") for h in range(2)]
                for pj in range(N_PJ):
                    for h in range(2):
                        nc.tensor.matmul(
                            acc0[h][:],
                            xbd[:, 128 * pj:128 * pj + 128],
                            wp[:, OD * pj + 512 * h:OD * pj + 512 * h + 512],
                            start=(pj == 0), stop=(pj == N_PJ - 1),
                        )
                # fold the two capsule slots (partitions 0-63 + 64-127)
                s0 = small.tile([64, OD], f32, tag="sfold")
                tmpu = small.tile([64, OD], f32, tag="tmpu")
                for h in range(2):
                    th = work.tile([128, 512], f32, tag="thfold", bufs=2)
                    nc.scalar.copy(out=th[:], in_=acc0[h][:])
                    nc.sync.dma_start(tmpu[:, 512 * h:512 * h + 512],
                                      th[64:128, :])
                    nc.vector.tensor_add(out=s0[:, 512 * h:512 * h + 512],
                                         in0=th[0:64, :],
                                         in1=tmpu[:, 512 * h:512 * h + 512])
            nc.scalar.mul(out=s0[:], in_=s0[:], mul=1.0 / NUM_CAPS)

            def all_reduce(sp):
                if stage == 4:  # timing variant: skip collectives
                    return sp
                k = ar_count[0]
                ar_count[0] += 1
                ar_in = dram.tile([64, OD], f32, name=f"ar_in{k}",
                                  tag=f"ar_in{k}")
                ar_out = dram.tile([64, OD], f32, name=f"ar_out{k}",
                                   tag=f"ar_out{k}")
                nc.sync.dma_start(ar_in[:], sp[:])
                nc.gpsimd.collective_compute(
                    "AllReduce", Alu.add,
                    replica_groups=[list(range(N_CORES))],
                    ins=[ar_in.opt()], outs=[ar_out.opt()])
                sq = small.tile([64, OD], f32, tag="sfold")
                nc.sync.dma_start(sq[:], ar_out[:])
                return sq

            def squash_to_vrep(sq):
                """v = (n/(1+n)) * s / sqrt(n+eps), n = sum_d s^2; replicate
                v into both partition halves of vrep, and build vrep32."""
                ssq = small.tile([64, OD], f32, tag="tmpu")
                nc.vector.tensor_mul(out=ssq[:], in0=sq[:], in1=sq[:])
                n_t = small.tile([64, NUM_CAPS], f32, tag="n_t")
                nc.vector.tensor_reduce(
                    out=n_t[:],
                    in_=bass.AP(tensor=ssq.tensor, offset=ssq[:].offset,
                                ap=[list(ssq[:].ap[0]), [1, NUM_CAPS],
                                    [NUM_CAPS, DIM_CAPS]]),
                    axis=AxX, op=Alu.add)
                sr = small.tile([64, NUM_CAPS], f32, tag="sr")
                # sqrt via exp(0.5*ln): Ln/Exp share an ACT table set
                nc.scalar.activation(out=sr[:], in_=n_t[:], func=Act.Ln,
                                     bias=eps_t[:], scale=1.0)
                nc.scalar.activation(out=sr[:], in_=sr[:], func=Act.Exp,
                                     bias=0.0, scale=0.5)
                nc.vector.reciprocal(out=sr[:], in_=sr[:])   # 1/sqrt(n+eps)
                np1 = small.tile([64, NUM_CAPS], f32, tag="np1")
                nc.vector.tensor_scalar_add(out=np1[:], in0=n_t[:],
                                            scalar1=1.0)
                nc.vector.reciprocal(out=np1[:], in_=np1[:])  # 1/(1+n)
                fac = small.tile([64, NUM_CAPS], f32, tag="fac")
                nc.vector.tensor_mul(out=fac[:], in0=n_t[:], in1=np1[:])
                nc.vector.tensor_mul(out=fac[:], in0=fac[:], in1=sr[:])
                for half in range(2):
                    nc.vector.tensor_tensor(
                        out=vrep2[0:64, OD * half:OD * half + OD].rearrange(
                            "p (d o) -> p d o", d=DIM_CAPS),
                        in0=sq[:].rearrange("p (d o) -> p d o", d=DIM_CAPS),
                        in1=bass.AP(tensor=fac.tensor, offset=fac[:].offset,
                                    ap=[list(fac[:].ap[0]), [0, DIM_CAPS],
                                        [1, NUM_CAPS]]),
                        op=Alu.mult)
                nc.vector.tensor_scalar_mul(out=vrep32_2[0:64, :],
                                            in0=vrep2[0:64, :], scalar1=32.0)
                nc.sync.dma_start(vrep2[64:128, :], vrep2[0:64, :])
                nc.sync.dma_start(vrep32_2[64:128, :], vrep32_2[0:64, :])

            sq = all_reduce(s0)
            squash_to_vrep(sq)

            # ---------------- iterations 1..2 -----------------------------
            with tc.tile_pool(name="ps", bufs=2, space="PSUM") as ps:
                for it in range(1, ROUTING_ITERS):
                    accps = ps.tile([64, OD], f32, name="accps",
                                    tag="accps", bufs=1)
                    acc_started = [False, False]
                    cm_ready = []   # previous batch's cm tiles, safe to fold

                    def emit_cmacc(last=False):
                        for j, cmr in enumerate(cm_ready):
                            for s_ in range(2):
                                for h in range(2):
                                    st = not acc_started[h]
                                    acc_started[h] = True
                                    nc.tensor.matmul(
                                        accps[:, 512 * h:512 * h + 512],
                                        foldF[:],
                                        cmr[:, 1024 * s_ + 512 * h:
                                            1024 * s_ + 512 * h + 512],
                                        start=st,
                                        stop=(last and
                                              j == len(cm_ready) - 1 and
                                              s_ == 1),
                                        skip_group_check=True,
                                    )
                        cm_ready.clear()

                    def emit_agr_folds(st):
                        """d-reduce each m of a finished batch into agrps."""
                        agrps = st["agrps"]
                        # fold DVE-produced m first; GPSIMD m lands latest
                        for sub, m, route in sorted(
                                st["m"], key=lambda x: x[2] == "p8"):
                            for s_ in range(2):
                                osl = agrps[:, 64 * sub + 32 * s_:
                                            64 * sub + 32 * s_ + 32]
                                if route == "d16":
                                    for k in range(32):
                                        nc.tensor.matmul(
                                            osl, ident[:],
                                            m[:, 1024 * s_ + 32 * k:
                                              1024 * s_ + 32 * k + 32],
                                            start=(k == 0), stop=(k == 31),
                                            skip_group_check=True)
                                else:
                                    for k in range(16):
                                        rhs = bass.AP(
                                            tensor=m.tensor,
                                            offset=m[:, 1024 * s_ +
                                                     64 * k:].offset,
                                            ap=[list(m[:].ap[0]), [32, 2],
                                                [1, 32]])
                                        nc.tensor.matmul(
                                            osl, identDR_ap, rhs,
                                            start=(k == 0), stop=(k == 15),
                                            perf_mode=DR,
                                            skip_group_check=True)

                    def emit_softmax_head(st):
                        """blx + exp for a finished batch (emitted early so
                        the ACT queue reaches exp before the evac burst)."""
                        batch = st["batch"]
                        agrps = st["agrps"]
                        blsl = bl[:, 256 * batch:256 * (batch + 1)]
                        ce = work.tile([128, 256], bf16, tag="ce", bufs=2)
                        if it == 1:
                            # logits = agreement (b was zero)
                            nc.scalar.activation(out=ce[:], in_=agrps[:],
                                                 func=Act.Exp, bias=0.0,
                                                 scale=1.0)
                            nc.scalar.copy(out=blsl, in_=agrps[:])
                        else:
                            blx = work.tile([128, 256], bf16, tag="blx",
                                            bufs=2)
                            nc.vector.scalar_tensor_tensor(
                                out=blx[:], in0=agrps[:], scalar=1.0,
                                in1=blsl, op0=Alu.mult, op1=Alu.add)
                            nc.scalar.activation(out=ce[:], in_=blx[:],
                                                 func=Act.Exp, bias=0.0,
                                                 scale=1.0)
                        st["ce"] = ce

                    def emit_softmax_cm(st):
                        """Softmax tail + c*u for a finished batch; folds the
                        still-older batch's cm tiles on the PE."""
                        ce = st["ce"]
                        zs = work.tile([128, 8], f32, tag="zs", bufs=4)
                        nc.vector.tensor_reduce(
                            out=zs[:],
                            in_=ce[:].rearrange("p (s o) -> p s o", s=8),
                            axis=AxX, op=Alu.add)
                        zr = work.tile([128, 8], bf16, tag="zr", bufs=4)
                        nc.vector.reciprocal(out=zr[:], in_=zs[:])
                        nc.vector.tensor_tensor(
                            out=ce[:].rearrange("p (s o) -> p s o", s=8),
                            in0=ce[:].rearrange("p (s o) -> p s o", s=8),
                            in1=bass.AP(tensor=zr.tensor, offset=zr[:].offset,
                                        ap=[list(zr[:].ap[0]), [1, 8],
                                            [0, NUM_CAPS]]),
                            op=Alu.mult)
                        # fold the two-batches-ago c*u into PSUM now (the
                        # in-order PE never stalls on fresh cm)
                        emit_cmacc()
                        for sub, ub in enumerate(st["ub"]):
                            csl = bass.AP(
                                tensor=ce.tensor,
                                offset=ce[:, 64 * sub:].offset,
                                ap=[list(ce[:].ap[0]), [NUM_CAPS, 2],
                                    [0, DIM_CAPS], [1, NUM_CAPS]])
                            cm = work.tile([128, 2 * OD], bf16, name="cm",
                                           tag="cm", bufs=8)
                            nc.vector.tensor_tensor(
                                out=cm[:].rearrange("p (s d o) -> p s d o",
                                                    s=2, d=DIM_CAPS),
                                in0=ub[:].rearrange(
                                    "p (s d o) -> p s d o", s=2, d=DIM_CAPS),
                                in1=csl, op=Alu.mult)
                            cm_ready.append(cm)

                    prev = None
                    for batch in range(N_BATCH):
                        st = {"batch": batch, "ub": [], "m": [],
                              "agrps": ps.tile([128, 256], f32, name="agrps",
                                               tag="agrps", bufs=2)}
                        for sub in range(4):
                            q = 4 * batch + sub
                            route = M_ROUTE_PAT[q % 16]
                            ub = work.tile([128, 2 * OD], bf16, tag="ub",
                                           bufs=10)
                            if it > 1:
                                # u is bit-identical across iterations:
                                # reload it from DRAM instead of recomputing
                                # on the PE and re-evacuating through ACT
                                nc.sync.dma_start(ub[:], u_dram[q][:])
                            else:
                                for pr in range(2):
                                    pair = 2 * q + pr
                                    rg, pj = pair % N_RG, pair // N_RG
                                    ups = ps.tile([128, OD], f32, name="ups",
                                                  tag="ups", bufs=2)
                                    for h in range(2):
                                        nc.tensor.matmul(
                                            ups[:, 512 * h:512 * h + 512],
                                            lhsT_of(xbd, rg, pj),
                                            rhs_of(wp, rg, pj, h),
                                            start=True, stop=True,
                                            tile_position=(32 * rg, 0),
                                        )
                                    dst = ub[:, OD * pr:OD * (pr + 1)]
                                    evac = EVAC_PAT[(2 * sub + pr) % 8]
                                    if evac == "act":
                                        nc.scalar.copy(out=dst, in_=ups[:])
                                    elif evac == "dve":
                                        nc.vector.tensor_copy(out=dst,
                                                              in_=ups[:])
                                    else:
                                        nc.gpsimd.tensor_copy(out=dst,
                                                              in_=ups[:])
                                nc.sync.dma_start(u_dram[q][:], ub[:])
                            # agreement products m = u*v (or u*32v for fp8)
                            if route == "d16":
                                m = work.tile([128, 2 * OD], bf16, tag="m16",
                                              bufs=3)
                                nc.vector.tensor_tensor(
                                    out=m[:], in0=ub[:], in1=vrep2[:],
                                    op=Alu.mult)
                            elif route == "d8":
                                m = work.tile([128, 2 * OD], f8, tag="m8",
                                              bufs=8)
                                nc.vector.tensor_tensor(
                                    out=m[:], in0=ub[:], in1=vrep32_2[:],
                                    op=Alu.mult)
                            else:
                                m = work.tile([128, 2 * OD], f8, tag="m8",
                                              bufs=8)
                                nc.gpsimd.tensor_tensor(
                                    out=m[:], in0=ub[:], in1=vrep32_2[:],
                                    op=Alu.mult)
                            st["m"].append((sub, m, route))
                            st["ub"].append(ub)
                        if prev is not None:
                            emit_agr_folds(prev)
                            emit_softmax_head(prev)
                            emit_softmax_cm(prev)
                        prev = st
                    emit_agr_folds(prev)
                    emit_softmax_head(prev)
                    emit_softmax_cm(prev)
                    emit_cmacc(last=True)
                    # evacuate the PSUM s-partial (i2 already folded)
                    acc = work.tile([64, OD], f32, tag="accev", bufs=1)
                    nc.scalar.copy(out=acc[:], in_=accps[:])
                    if it < ROUTING_ITERS - 1:
                        sq = all_reduce(acc)
                        squash_to_vrep(sq)
                    else:
                        nc.sync.dma_start(out_d[:], acc[:])
    nc.compile()
    return nc


def _prep_inputs(x, W):
    """Build per-core xbd [128, N_PJ*128] and wp [128, N_PJ*OD] arrays."""
    import ml_dtypes
    bf16 = ml_dtypes.bfloat16
    ins = []
    for c in range(N_CORES):
        xc = x[:, c * I_LOC:(c + 1) * I_LOC, :]          # [64, 256, 16]
        Wc = W[c * I_LOC:(c + 1) * I_LOC]                # [256, 32, 32, 16]
        # i_loc = 8*pj + 2*rg + ipar
        xr = np.ascontiguousarray(
            xc.reshape(B, N_PJ, N_RG, 2, IN_DIM)
              .transpose(3, 2, 4, 1, 0))                 # [ipar,rg,p,pj,b]
        xbd = np.zeros((N_RG, 2, IN_DIM, N_PJ, 2, B), dtype=np.float32)
        xbd[:, 0, :, :, 0, :] = xr[0]
        xbd[:, 1, :, :, 1, :] = xr[1]
        xbd = xbd.reshape(128, N_PJ * 128).astype(bf16)
        wr = np.ascontiguousarray(
            Wc.reshape(N_PJ, N_RG, 2, NUM_CAPS, DIM_CAPS, IN_DIM)
              .transpose(1, 2, 5, 0, 4, 3)               # [rg,ipar,p,pj,d,o]
              .reshape(128, N_PJ * OD)).astype(bf16)
        ins.append({"xbd": xbd, "wp": wr})
    return ins


def _squash_np(s):
    n = np.sum(np.square(s), axis=-1, keepdims=True)
    return (n / (1.0 + n)) * (s / np.sqrt(n + EPS))


def kernel(x, W, _trace=False):
    from concourse.bass_utils import run_bass_kernel_spmd

    x = np.asarray(x, dtype=np.float32)
    W = np.asarray(W, dtype=np.float32)
    if "nc" not in _CACHE:
        _CACHE["nc"] = _build_nc()
    nc = _CACHE["nc"]
    in_maps = _prep_inputs(x, W)
    res = run_bass_kernel_spmd(nc, in_maps, core_ids=list(range(N_CORES)),
                               trace=_trace)
    _CACHE["last_result"] = res
    sp = np.stack([r["out_sp2"] for r in res.results])   # [8, 64, OD]
    s2 = sp.sum(axis=0)
    s2_od = s2.reshape(B, DIM_CAPS, NUM_CAPS).transpose(0, 2, 1)
    v = _squash_np(np.ascontiguousarray(s2_od))
    return v.astype(np.float32)
